# revision 44
# baseline (speedup 1.0000x reference)
"""Trainium2 Bass kernel for BitNet multi-head attention (nn_MultiHeadAttention_62294205661880).

Sharding: 8 cores = 2 batches x 4 head-groups (4 heads each).  Each core
computes qkv projection, RoPE, causal attention and a column-parallel slice
of the output projection for its (batch, head-group); the host sums the 4
partial out-projections per batch (the tensor-parallel all-reduce done
host-side, since the contract gathers to host anyway).

BitNet quantization is folded on the host: weights are uploaded as exact
ternary {-1,0,+1} matrices; scale_qkv^2/sqrt(dh) is folded into the
softmax exp() scale and scale_qkv*scale_out into a final host-side scalar.

Device layout: everything is computed transposed.  Q_T/K_T come out of
the projection as [dh, S]; scores are computed as s_T[k, q]; the softmax
denominator sums over the partition (key) dim via DVE fp16 accumulation of
the exp() blocks followed by a small all-ones stationary matmul (which also
replicates the sums across partitions for free); AV produces out_T[dh, q]
which feeds the output projection directly.  No on-device transposes.

Speed tricks vs the bf16 version:
  * Q/K projections run in fp8 (e4m3) DoubleRow mode: the ternary weights
    are exact in fp8 and x is quantized to e4m3 host-side; the softmax
    washes the ~2.6% x-quantization noise out (measured end-to-end rel err
    ~1e-2 vs the 2e-2 budget).  V stays bf16 (its error passes straight
    through to the output).
  * The softmax-denominator matmul-per-block is replaced by DVE fp16
    pair-accumulation + 2 matmuls per (head, query-chunk).
  * exp() is evaluated once per key-block PAIR (halves ACT instruction
    overhead).
"""

import sys
import types

import numpy as np
import ml_dtypes

import concourse.bass as bass
import concourse.mybir as mybir
import concourse.tile as tile
from concourse import bacc
from concourse.bass_utils import run_bass_kernel_spmd

D_MODEL = 2048
N_HEADS = 16
D_HEAD = 128
SEQ = 2048
BATCH = 2
ROPE_BASE = 10000.0

N_CORES = 8
HPC = 4  # heads per core
R_LOCAL = HPC * D_HEAD  # 512 local q (or k, or v) rows per core
MO = D_MODEL // 128  # 16 contraction blocks
MP = MO // 2  # 8 contraction block-pairs (DoubleRow)
NKI = SEQ // 128  # 16 key blocks
NQC = SEQ // 512  # 4 query chunks of 512
NSB = SEQ // 128  # 16 seq blocks (v / proj)

BF16 = mybir.dt.bfloat16
F16 = mybir.dt.float16
F32 = mybir.dt.float32
NPBF16 = ml_dtypes.bfloat16
NPFP8 = ml_dtypes.float8_e4m3
FP8 = mybir.dt.float8e4
DR = mybir.MatmulPerfMode.DoubleRow

LAST_RESULT = None  # BassKernelResults of the most recent run (for test.py)
_PROG_CACHE = {}
PROFILE = False  # test.py sets True to capture an NTFF profile / HW exec time


def _enable_profiling() -> bool:
    """Install the axon NTFF profile hook glue if the image lacks
    ``antenv.axon_hooks`` (boot degrades silently without it), and skip
    the artifact upload (no bucket access in this container)."""
    try:
        from antenv.axon_hooks import get_axon_ntff_profile_hook  # noqa: F401

        ok = get_axon_ntff_profile_hook() is not None
    except ImportError:
        ok = False
        import antenv

        mod = types.ModuleType("antenv.axon_hooks")
        mod._hook = None
        mod.set_axon_ntff_profile_hook = lambda h: setattr(mod, "_hook", h)
        mod.get_axon_ntff_profile_hook = lambda: mod._hook
        sys.modules["antenv.axon_hooks"] = mod
        antenv.axon_hooks = mod
        try:
            from trn_agent_boot.trn_boot import _ntff_profile_via_ctypes

            hook = _ntff_profile_via_ctypes("/opt/axon/libaxon_pjrt.so")
            if hook is not None:
                mod._hook = hook
                ok = True
        except Exception as e:  # profiling is best-effort
            print(f"ntff profile hook install failed: {e}", file=sys.stderr)
    if ok:
        import concourse.bass_utils as _bu

        _bu.upload_artifacts = lambda tmpdir: tmpdir
    return ok


def _build_program(causal: bool, exp_scale: float) -> bass.Bass:
    nc = bacc.Bacc(None)
    S = SEQ

    # weight/x layouts are pre-swizzled on the host to the exact SBUF layout
    # (partition-major) so each load is one large fully-contiguous DMA
    xTs_d = nc.dram_tensor("xTs", [4 * 128, MO * 512], BF16, kind="ExternalInput")
    x8_d = nc.dram_tensor("x8", [D_MODEL, S], FP8, kind="ExternalInput")
    wqT_d = nc.dram_tensor("wqT", [128, MO * R_LOCAL], FP8, kind="ExternalInput")
    wkT_d = nc.dram_tensor("wkT", [128, MO * R_LOCAL], FP8, kind="ExternalInput")
    wvT_d = nc.dram_tensor("wvT", [128, MO * R_LOCAL], FP8, kind="ExternalInput")
    woT_d = nc.dram_tensor("woT", [128, HPC * D_MODEL], BF16, kind="ExternalInput")
    # cos rows 0:64, sin rows 64:128
    cs_d = nc.dram_tensor("cossinT", [128, S], BF16, kind="ExternalInput")
    # swapped: sin rows 0:64, cos rows 64:128 (keeps TensorTensor base partitions equal)
    sc_d = nc.dram_tensor("sincosT", [128, S], BF16, kind="ExternalInput")
    if causal:
        # 16 transposed diagonal 128x128 mask blocks, side by side
        maskd_d = nc.dram_tensor("maskd", [128, S], BF16, kind="ExternalInput")
    else:
        maskf_d = nc.dram_tensor("maskf", [S, S], BF16, kind="ExternalInput")
    out_d = nc.dram_tensor("out", [S, D_MODEL], BF16, kind="ExternalOutput")

    x8_v = x8_d[:].rearrange("(mo p) s -> p mo s", p=128)
    wqT_v = wqT_d[:].rearrange("p (mo r) -> p mo r", mo=MO)
    wkT_v = wkT_d[:].rearrange("p (mo r) -> p mo r", mo=MO)
    if not causal:
        maskf_v = maskf_d[:].rearrange("(ko p) q -> p ko q", p=128)

    with tile.TileContext(nc) as tc:
        with tc.tile_pool(name="pers", bufs=1) as pers:
            # ---- persistent SBUF tensors (live across both phases) ----
            q_rot = pers.tile([128, HPC, S], BF16, tag="qrot")
            k_rot = pers.tile([128, HPC, S], BF16, tag="krot")
            v_sb = pers.tile([128, NKI, R_LOCAL], BF16, tag="vsb")
            ones16 = pers.tile([128, 128], F16, tag="ones")
            warm = pers.tile([128, 1], BF16, tag="warm")
            if causal:
                maskd = pers.tile([128, S], BF16, tag="maskd")
            nc.vector.memset(ones16[:, :], 1.0)

            # ================= phase A: QKV projection + RoPE =================
            with (
                tc.tile_pool(name="x8p", bufs=1) as x8p,
                tc.tile_pool(name="wp", bufs=1) as wp,
                tc.tile_pool(name="raw", bufs=3) as rawp,
                tc.tile_pool(name="w8", bufs=2) as w8p,
                tc.tile_pool(name="tmp", bufs=4) as tmpp,
                tc.tile_pool(name="xvp", bufs=2) as xvp,
                tc.tile_pool(name="psA", bufs=2, space="PSUM") as psA,
            ):
                x8 = x8p.tile([128, MO, S], FP8, tag="x8")
                wq8 = wp.tile([128, MO, R_LOCAL], FP8, tag="wq8")
                wk8 = wp.tile([128, MO, R_LOCAL], FP8, tag="wk8")
                wv = wp.tile([128, MO, R_LOCAL], BF16, tag="wv")
                cs_t = wp.tile([128, S], BF16, tag="cs")
                sc_t = wp.tile([128, S], BF16, tag="sc")

                # batched DMAs (large transfers amortize per-trigger cost) with
                # small leading slices so head-0's first m-pairs start early;
                # x8 paces head-0 so it rides its own queue ring (sync)
                # while the weights go on the scalar ring
                # head-0 is paced by the x8 stream: alternate blocks across
                # the two DMA queue rings so both DMA engines feed it in
                # parallel; the q/k weight heads ride ahead on the scalar ring
                nc.scalar.dma_start(out=x8[:, 1:2, :], in_=x8_v[:, 1:2, :])
                nc.scalar.dma_start(out=wq8[:, 0:2, :], in_=wqT_v[:, 0:2, :])
                nc.scalar.dma_start(out=wk8[:, 0:2, :], in_=wkT_v[:, 0:2, :])
                for mo in range(MO):
                    if mo == 1:
                        continue  # issued first on the scalar ring above
                    eng = nc.sync if mo % 2 == 0 else nc.scalar
                    eng.dma_start(out=x8[:, mo : mo + 1, :], in_=x8_v[:, mo : mo + 1, :])
                nc.sync.dma_start(out=wq8[:, 2:16, :], in_=wqT_v[:, 2:16, :])
                nc.scalar.dma_start(out=wk8[:, 2:16, :], in_=wkT_v[:, 2:16, :])
                nc.sync.dma_start(out=cs_t[:, :], in_=cs_d[:, :])
                nc.scalar.dma_start(out=sc_t[:, :], in_=sc_d[:, :])
                if causal:
                    nc.sync.dma_start(out=maskd[:, :], in_=maskd_d[:, :])
                # V weights arrive fp8, converted to bf16 on DVE
                for g in range(2):
                    st = w8p.tile([128, 8 * R_LOCAL], FP8, tag="w8")
                    eng = nc.scalar if g % 2 == 0 else nc.sync
                    eng.dma_start(
                        out=st[:, :],
                        in_=wvT_d[:, g * 8 * R_LOCAL : (g + 1) * 8 * R_LOCAL],
                    )
                    nc.vector.tensor_copy(
                        wv[:, 8 * g : 8 * g + 8, :],
                        st[:].rearrange("p (mo r) -> p mo r", mo=8),
                    )
                # load the exp table set now (before phase B) so no ACT table
                # switch happens mid-kernel; emitted after the DMA triggers so
                # the ~2.7us table load does not delay the weight streams
                nc.scalar.activation(
                    warm[:, :], ones16[:, 0:1], mybir.ActivationFunctionType.Exp
                )

                def rope(dst, raw):
                    """NeoX rotary: rows 0:64 = t*c - b*s ; rows 64:128 = t*s + b*c."""
                    ta = tmpp.tile([64, S], BF16, tag="tmp")
                    tb = tmpp.tile([64, S], BF16, tag="tmp")
                    nc.vector.tensor_mul(ta[:, :], raw[0:64, :], cs_t[0:64, :])
                    nc.vector.tensor_mul(tb[:, :], raw[64:128, :], cs_t[64:128, :])
                    nc.vector.tensor_sub(dst[0:64, :], ta[:, :], tb[:, :])
                    tc2 = tmpp.tile([64, S], BF16, tag="tmp")
                    td = tmpp.tile([64, S], BF16, tag="tmp")
                    nc.vector.tensor_mul(tc2[:, :], raw[0:64, :], sc_t[0:64, :])
                    nc.vector.tensor_mul(td[:, :], raw[64:128, :], sc_t[64:128, :])
                    nc.vector.tensor_add(dst[64:128, :], tc2[:, :], td[:, :])

                # head 0 q/k with the m-pair loop OUTER so the DR matmuls
                # consume x8 m-blocks as the DMAs land (startup overlap).
                qp0 = psA.tile([128, S], F32, tag="psA")
                kp0 = psA.tile([128, S], F32, tag="psA")
                for mp in range(MP):
                    for c4 in range(4):
                        nc.tensor.matmul(
                            qp0[:, c4 * 512 : (c4 + 1) * 512],
                            wq8[:, 2 * mp : 2 * mp + 2, 0:128],
                            x8[:, 2 * mp : 2 * mp + 2, c4 * 512 : (c4 + 1) * 512],
                            start=(mp == 0),
                            stop=(mp == MP - 1),
                            perf_mode=DR,
                        )
                        nc.tensor.matmul(
                            kp0[:, c4 * 512 : (c4 + 1) * 512],
                            wk8[:, 2 * mp : 2 * mp + 2, 0:128],
                            x8[:, 2 * mp : 2 * mp + 2, c4 * 512 : (c4 + 1) * 512],
                            start=(mp == 0),
                            stop=(mp == MP - 1),
                            perf_mode=DR,
                        )
                q_raw = rawp.tile([128, S], BF16, tag="raw")
                nc.scalar.copy(q_raw[:, :], qp0[:, :])
                rope(q_rot[:, 0, :], q_raw)
                k_raw = rawp.tile([128, S], BF16, tag="raw")
                nc.scalar.copy(k_raw[:, :], kp0[:, :])
                rope(k_rot[:, 0, :], k_raw)

                def project_dr(dst_raw, w_sb, h):
                    """q/k head projection (fp8 DoubleRow) -> bf16 raw [128, S]."""
                    ps = psA.tile([128, S], F32, tag="psA")
                    for c4 in range(4):
                        for mp in range(MP):
                            nc.tensor.matmul(
                                ps[:, c4 * 512 : (c4 + 1) * 512],
                                w_sb[:, 2 * mp : 2 * mp + 2, h * 128 : (h + 1) * 128],
                                x8[:, 2 * mp : 2 * mp + 2, c4 * 512 : (c4 + 1) * 512],
                                start=(mp == 0),
                                stop=(mp == MP - 1),
                                perf_mode=DR,
                            )
                    nc.scalar.copy(dst_raw[:, :], ps[:, :])

                for h in range(1, HPC):
                    q_raw = rawp.tile([128, S], BF16, tag="raw")
                    project_dr(q_raw, wq8, h)
                    rope(q_rot[:, h, :], q_raw)
                    k_raw = rawp.tile([128, S], BF16, tag="raw")
                    project_dr(k_raw, wk8, h)
                    rope(k_rot[:, h, :], k_raw)

                # V projection (bf16; natural layout [s, r]); x seq-slices are
                # DMA-streamed per 512-col group, 4 seq blocks per psum tile
                for sb4 in range(NSB // 4):
                    ps = psA.tile([128, S], F32, tag="psA")
                    xv = xvp.tile([128, MO, 512], BF16, tag="xv")
                    xv_view = xTs_d[sb4 * 128 : (sb4 + 1) * 128, :].rearrange(
                        "p (mo c) -> p mo c", mo=MO
                    )
                    # two half-DMAs on alternating rings; the m-outer matmul
                    # loop starts on the first half while the second streams
                    eng0 = nc.sync if sb4 % 2 == 0 else nc.scalar
                    eng1 = nc.scalar if sb4 % 2 == 0 else nc.sync
                    eng0.dma_start(out=xv[:, 0:8, :], in_=xv_view[:, 0:8, :])
                    eng1.dma_start(out=xv[:, 8:16, :], in_=xv_view[:, 8:16, :])
                    for m in range(MO):
                        for part in range(4):
                            nc.tensor.matmul(
                                ps[:, part * 512 : part * 512 + 512],
                                xv[:, m, part * 128 : (part + 1) * 128],
                                wv[:, m, :],
                                start=(m == 0),
                                stop=(m == MO - 1),
                            )
                    nc.scalar.copy(v_sb[:, sb4 * 4 : sb4 * 4 + 4, :], ps[:, :])

            # ================= phase B: attention + out-projection =============
            with (
                tc.tile_pool(name="wop", bufs=1) as wop,
                tc.tile_pool(name="pp", bufs=6) as ppp,
                tc.tile_pool(name="accp", bufs=2) as accp,
                tc.tile_pool(name="rcp", bufs=2) as rcp,
                tc.tile_pool(name="aop", bufs=2) as aop,
                tc.tile_pool(name="osb", bufs=4) as osbp,
                tc.tile_pool(name="mblk", bufs=4) as mblkp,
                tc.tile_pool(name="sp", bufs=2, space="PSUM") as spp,
                tc.tile_pool(name="op", bufs=1, space="PSUM") as opp,
                tc.tile_pool(name="avs", bufs=2, space="PSUM") as avsp,
            ):
                wo = wop.tile([128, HPC, D_MODEL], BF16, tag="wo")
                nc.scalar.dma_start(
                    out=wo[:, :, :],
                    in_=woT_d[:, :].rearrange("p (h o) -> p h o", h=HPC),
                )

                # out-projection chunks of query-chunk qc are emitted
                # interleaved into qc+1's attention stream so the tensor
                # engine has ready work whenever an AV matmul would stall
                # on its exp()
                pending = []
                evict_ctr = [0]
                # each head's finalize (sums matmuls -> recip -> normalize) is
                # deferred until after the next head's lookahead scores so it
                # never head-of-line-blocks the tensor queue
                pending_final = [None]

                def emit_op_chunk(pool=None):
                    pending.pop(0)(pool)

                for qc in range(NQC):
                    q_lo = qc * 512
                    nki_here = (4 * qc + 4) if causal else NKI
                    npair = nki_here // 2
                    spc = max(1, (HPC * npair) // 8)
                    slot = [0]

                    def maybe_op():
                        slot[0] += 1
                        if pending and slot[0] % spc == 0:
                            emit_op_chunk()

                    aoq = aop.tile([128, HPC, 512], BF16, tag="ao")
                    for h in range(HPC):
                        avs = avsp.tile([128, 512], F32, tag="avs")
                        # SBUF fp16 accumulator for the softmax denominator:
                        # [0:512] even key-blocks, [512:1024] odd key-blocks
                        acc = accp.tile([128, 1024], F16, tag="acc")

                        def pair_params(pr):
                            ki0, ki1 = 2 * pr, 2 * pr + 1
                            d0 = causal and ki0 >= 4 * qc
                            d1 = causal and ki1 >= 4 * qc
                            q00 = 128 * (ki0 - 4 * qc) if d0 else 0
                            q01 = 128 * (ki1 - 4 * qc) if d1 else 0
                            return ki0, ki1, d0, d1, q00, q01

                        def emit_scores(pr):
                            """scores pair -> exp -> mask -> denominator acc;
                            returns the pp tile for the deferred AV."""
                            ki0, ki1, d0, d1, q00, q01 = pair_params(pr)
                            spb = spp.tile([128, 1024], F32, tag="sp")
                            pp = ppp.tile([128, 1024], BF16, tag="pp")
                            nc.tensor.matmul(
                                spb[:, q00:512],
                                k_rot[:, h, ki0 * 128 : (ki0 + 1) * 128],
                                q_rot[:, h, q_lo + q00 : q_lo + 512],
                                start=True,
                                stop=True,
                            )
                            nc.tensor.matmul(
                                spb[:, 512 + q01 : 1024],
                                k_rot[:, h, ki1 * 128 : (ki1 + 1) * 128],
                                q_rot[:, h, q_lo + q01 : q_lo + 512],
                                start=True,
                                stop=True,
                            )
                            if q00 == 0 and q01 == 0:
                                nc.scalar.activation(
                                    pp[:, 0:1024],
                                    spb[:, 0:1024],
                                    mybir.ActivationFunctionType.Exp,
                                    scale=float(exp_scale),
                                )
                            else:
                                nc.scalar.activation(
                                    pp[:, q00:512],
                                    spb[:, q00:512],
                                    mybir.ActivationFunctionType.Exp,
                                    scale=float(exp_scale),
                                )
                                nc.scalar.activation(
                                    pp[:, 512 + q01 : 1024],
                                    spb[:, 512 + q01 : 1024],
                                    mybir.ActivationFunctionType.Exp,
                                    scale=float(exp_scale),
                                )
                            if d0:
                                nc.vector.tensor_mul(
                                    pp[:, q00 : q00 + 128],
                                    pp[:, q00 : q00 + 128],
                                    maskd[:, ki0 * 128 : (ki0 + 1) * 128],
                                )
                            if d1:
                                nc.vector.tensor_mul(
                                    pp[:, 512 + q01 : 512 + q01 + 128],
                                    pp[:, 512 + q01 : 512 + q01 + 128],
                                    maskd[:, ki1 * 128 : (ki1 + 1) * 128],
                                )
                            if not causal:
                                mb = mblkp.tile([128, 1024], BF16, tag="mblk")
                                nc.sync.dma_start(
                                    out=mb[:, 0:512],
                                    in_=maskf_v[:, ki0, q_lo : q_lo + 512],
                                )
                                nc.sync.dma_start(
                                    out=mb[:, 512:1024],
                                    in_=maskf_v[:, ki1, q_lo : q_lo + 512],
                                )
                                nc.vector.tensor_mul(
                                    pp[:, 0:1024], pp[:, 0:1024], mb[:, :]
                                )
                            # softmax denominator: DVE fp16 pair accumulation
                            if pr == 0:
                                if q01 == 0:
                                    nc.vector.tensor_copy(
                                        acc[:, 0:1024], pp[:, 0:1024]
                                    )
                                else:  # qc == 0: ki0 full, ki1 starts at 128
                                    nc.vector.tensor_copy(
                                        acc[:, 0:512], pp[:, 0:512]
                                    )
                                    nc.vector.tensor_copy(
                                        acc[:, 512 + q01 : 1024],
                                        pp[:, 512 + q01 : 1024],
                                    )
                            elif q00 == 0 and q01 == 0:
                                nc.vector.tensor_add(
                                    acc[:, 0:1024], acc[:, 0:1024], pp[:, 0:1024]
                                )
                            else:
                                nc.vector.tensor_add(
                                    acc[:, q00:512],
                                    acc[:, q00:512],
                                    pp[:, q00:512],
                                )
                                nc.vector.tensor_add(
                                    acc[:, 512 + q01 : 1024],
                                    acc[:, 512 + q01 : 1024],
                                    pp[:, 512 + q01 : 1024],
                                )
                            return pp

                        LOOKAHEAD = 2
                        pps = []
                        for p in range(min(LOOKAHEAD, npair)):
                            pps.append(emit_scores(p))
                        if pending_final[0] is not None:
                            pending_final[0]()
                            pending_final[0] = None
                        for pr in range(npair):
                            ki0, ki1, d0, d1, q00, q01 = pair_params(pr)
                            pp = pps[pr]
                            nc.tensor.matmul(
                                avs[:, q00:512],
                                v_sb[:, ki0, h * 128 : (h + 1) * 128],
                                pp[:, q00:512],
                                start=(pr == 0),
                                stop=False,
                            )
                            nc.tensor.matmul(
                                avs[:, q01:512],
                                v_sb[:, ki1, h * 128 : (h + 1) * 128],
                                pp[:, 512 + q01 : 1024],
                                start=False,
                                stop=(pr == npair - 1),
                            )
                            maybe_op()
                            if pr + LOOKAHEAD < npair:
                                pps.append(emit_scores(pr + LOOKAHEAD))
                        def finalize(h=h, acc=acc, avs=avs, aoq=aoq, qc=qc):
                            # partition-reduce the fp16 accumulator
                            # (replicated sums), then normalize
                            sums = opp.tile([128, 1024], F32, tag="op")
                            if causal and qc == 0:
                                # odd key-blocks contribute nothing for q < 128
                                # (acc[512:640] was never written)
                                nc.tensor.matmul(
                                    sums[:, 0:512],
                                    ones16[:, :],
                                    acc[:, 0:512],
                                    start=True,
                                    stop=False,
                                )
                                nc.tensor.matmul(
                                    sums[:, 128:512],
                                    ones16[:, :],
                                    acc[:, 512 + 128 : 1024],
                                    start=False,
                                    stop=True,
                                )
                            else:
                                nc.tensor.matmul(
                                    sums[:, 0:512],
                                    ones16[:, :],
                                    acc[:, 0:512],
                                    start=True,
                                    stop=False,
                                )
                                nc.tensor.matmul(
                                    sums[:, 0:512],
                                    ones16[:, :],
                                    acc[:, 512:1024],
                                    start=False,
                                    stop=True,
                                )
                            rc = rcp.tile([128, 512], F32, tag="rc")
                            nc.vector.reciprocal_approx_fast(
                                rc[:, :], sums[:, 0:512]
                            )
                            nc.vector.tensor_mul(
                                aoq[:, h, :], avs[:, 0:512], rc[:, :]
                            )

                        pending_final[0] = finalize

                    # queue this chunk's out-projection (4 seq blocks x 2
                    # column halves); emitted during the next chunk
                    for sb_rel in range(4):
                        for oc2 in range(2):

                            def op_chunk(pool, sb_rel=sb_rel, oc2=oc2, aoq=aoq, qc=qc):
                                sb = 4 * qc + sb_rel
                                if pool is None:
                                    op2 = opp.tile([128, 1024], F32, tag="op")
                                else:
                                    op2 = pool.tile([128, 1024], F32, tag="sp")
                                for h2 in range(HPC):
                                    lhsT = aoq[
                                        :, h2, sb_rel * 128 : (sb_rel + 1) * 128
                                    ]
                                    nc.tensor.matmul(
                                        op2[:, 0:512],
                                        lhsT,
                                        wo[:, h2, oc2 * 1024 : oc2 * 1024 + 512],
                                        start=(h2 == 0),
                                        stop=(h2 == HPC - 1),
                                    )
                                    nc.tensor.matmul(
                                        op2[:, 512:1024],
                                        lhsT,
                                        wo[:, h2, oc2 * 1024 + 512 : (oc2 + 1) * 1024],
                                        start=(h2 == 0),
                                        stop=(h2 == HPC - 1),
                                    )
                                # evicts mostly on DVE; every 3rd on ACT
                                ob = osbp.tile([128, 1024], BF16, tag="osb")
                                if evict_ctr[0] % 3 == 2:
                                    nc.scalar.copy(ob[:, :], op2[:, :])
                                else:
                                    nc.vector.tensor_copy(ob[:, :], op2[:, :])
                                evict_ctr[0] += 1
                                nc.sync.dma_start(
                                    out=out_d[
                                        sb * 128 : (sb + 1) * 128,
                                        oc2 * 1024 : (oc2 + 1) * 1024,
                                    ],
                                    in_=ob[:, :],
                                )

                            pending.append(op_chunk)

                if pending_final[0] is not None:
                    pending_final[0]()
                # final drain: alternate psum pools so consecutive chunks
                # do not serialize on a single tile's evict
                drain_flip = 0
                while pending:
                    emit_op_chunk(spp if drain_flip % 2 else None)
                    drain_flip += 1

    nc.finalize()
    return nc


def _bit_quantize_ternary(w: np.ndarray):
    """Returns (ternary {-1,0,1} float32 matrix, scale) matching the reference."""
    scale = np.maximum(np.mean(np.abs(w.astype(np.float32))), np.float32(1e-5))
    t = np.clip(np.round(w.astype(np.float32) / scale), -1.0, 1.0).astype(np.float32)
    return t, float(scale)


def _host_tables():
    """cos/sin stacked [128, S]: rows 0:64 cos, rows 64:128 sin."""
    inv_freq = 1.0 / (ROPE_BASE ** (np.arange(0, D_HEAD, 2, dtype=np.float32) / D_HEAD))
    pos = np.arange(SEQ, dtype=np.float32)
    ang = pos[:, None] * inv_freq[None, :]  # [S, 64]
    cs = np.empty((128, SEQ), dtype=NPBF16)
    cs[0:64] = np.ascontiguousarray(np.cos(ang).T).astype(NPBF16)
    cs[64:128] = np.ascontiguousarray(np.sin(ang).T).astype(NPBF16)
    sc = np.empty((128, SEQ), dtype=NPBF16)
    sc[0:64] = cs[64:128]
    sc[64:128] = cs[0:64]
    return cs, sc


def kernel(x, w_qkv, w_out, mask):
    global LAST_RESULT
    x = np.asarray(x, dtype=np.float32)
    w_qkv = np.asarray(w_qkv, dtype=np.float32)
    w_out = np.asarray(w_out, dtype=np.float32)
    mask = np.asarray(mask)

    tq, sq = _bit_quantize_ternary(w_qkv)
    to, so = _bit_quantize_ternary(w_out)
    exp_scale = (sq * sq) / float(np.sqrt(D_HEAD))
    c2 = np.float32(sq * so)

    m2 = (mask.reshape(SEQ, SEQ) != 0).astype(np.float32)
    causal = bool(np.array_equal(m2, np.tril(np.ones((SEQ, SEQ), np.float32))))

    cs, sc = _host_tables()
    if causal:
        maskd = np.empty((128, SEQ), dtype=NPBF16)
        for ki in range(NKI):
            blk = m2[ki * 128 : (ki + 1) * 128, ki * 128 : (ki + 1) * 128]  # [q, k]
            maskd[:, ki * 128 : (ki + 1) * 128] = np.ascontiguousarray(blk.T).astype(
                NPBF16
            )
    else:
        maskf = np.ascontiguousarray(m2.T).astype(NPBF16)  # [kk, qq]

    key = (causal, float(exp_scale))
    if key not in _PROG_CACHE:
        _PROG_CACHE[key] = _build_program(causal, float(exp_scale))
    nc = _PROG_CACHE[key]

    def swz_w(wT):
        """[D_MODEL, R] -> partition-major [128, MO*R] (SBUF layout)."""
        r = wT.shape[1]
        return np.ascontiguousarray(
            wT.reshape(MO, 128, r).transpose(1, 0, 2).reshape(128, MO * r)
        )

    in_maps = []
    for b in range(BATCH):
        xt_b = np.ascontiguousarray(x[b].T)  # [D_MODEL, S]
        x8_b = xt_b.astype(NPFP8)
        xts_b = np.ascontiguousarray(
            xt_b.reshape(MO, 128, 4, 512)
            .transpose(2, 1, 0, 3)
            .reshape(4 * 128, MO * 512)
        ).astype(NPBF16)
        for g in range(4):
            rows = slice(R_LOCAL * g, R_LOCAL * (g + 1))
            im = {
                "xTs": xts_b,
                "x8": x8_b,
                "wqT": swz_w(tq[0 * D_MODEL :][rows].T).astype(NPFP8),
                "wkT": swz_w(tq[1 * D_MODEL :][rows].T).astype(NPFP8),
                "wvT": swz_w(tq[2 * D_MODEL :][rows].T).astype(NPFP8),
                "woT": np.ascontiguousarray(
                    to[:, rows].T.reshape(HPC, 128, D_MODEL)
                    .transpose(1, 0, 2)
                    .reshape(128, HPC * D_MODEL)
                ).astype(NPBF16),
                "cossinT": cs,
                "sincosT": sc,
            }
            if causal:
                im["maskd"] = maskd
            else:
                im["maskf"] = maskf
            in_maps.append(im)

    do_trace = bool(PROFILE) and _enable_profiling()
    res = run_bass_kernel_spmd(nc, in_maps, list(range(N_CORES)), trace=do_trace)
    LAST_RESULT = res

    parts = [np.asarray(res.results[c]["out"]).astype(np.float32) for c in range(N_CORES)]
    out = np.stack(
        [
            parts[0] + parts[1] + parts[2] + parts[3],
            parts[4] + parts[5] + parts[6] + parts[7],
        ]
    )
    return (out * c2).astype(np.float32)
